# revision 13
# baseline (speedup 1.0000x reference)
"""BasicGCN (2-layer GCN, 100K nodes / 3.2M edges) on 8 Trainium2 NeuronCores.

Strategy (node/dst sharding, graph-parallel):
  - Pad nodes to NPAD = 100352 = 8 * 12544; core c owns dst rows
    [c*12544, (c+1)*12544).
  - Host preprocessing (index-space only): degrees/dinv, per-core edge
    streams sorted by (dst-tile, src-group), self-loops appended as real
    edges, per-(tile,group) slot quotas equalized across cores so one
    SPMD program serves all 8 cores.
  - Device per core:
      phase 1 (dense, replicated): h1p = dinv * (x @ W1) for ALL nodes,
        written to an HBM table [NPAD, 256] f32.
      phase 2 (layer-1 aggregation): for each 128-dst tile, dma_gather
        h1p rows for its edges (1KB/row), build one-hot S blocks on DVE
        (S[e,d] = dst_local[e]==d), segment-sum via PE matmul into PSUM;
        epilogue: out1 = dinv_d*agg + b1, relu, @W2 (via PE transpose),
        h2p_own = dinv_d * (relu(out1) @ W2) -> 12544x64 shard.
      AllGather shards -> h2p_full [NPAD, 64] (Shared DRAM).
      phase 3 (layer-2 aggregation): same gather/S/matmul with 256B rows,
        epilogue log_softmax -> out shard [12544, 64].
  - Host: concatenate 8 shards, trim to [100000, 64].

Gather tables are split into 4 row-groups of NPAD/4 = 25088 rows so the
int16 gather indices stay in range; the 4 groups of a dst tile are
separate dma_gather calls feeding the same PSUM accumulation.
"""

import numpy as np

import concourse.bacc as bacc
import concourse.bass as bass
import concourse.mybir as mybir
import concourse.tile as tile
from concourse.bass_utils import run_bass_kernel_spmd

F32 = mybir.dt.float32
I16 = mybir.dt.int16
AF = mybir.ActivationFunctionType
ALU = mybir.AluOpType

N_CORES = 8
PAD_DSTLOC = 1000.0  # sentinel dst-local for padding slots -> zero S column
QMAX = 1024  # max num_idxs per dma_gather call (HW limit is in (1024, 1280])


def _chunks(quota_row, ng):
    """Split each group's quota into <=QMAX chunks of whole 128-blocks."""
    out = []
    for g in range(ng):
        q = int(quota_row[g])
        off = 0
        while off < q:
            qc = min(QMAX, q - off)
            out.append((g, off, qc))
            off += qc
    return out


def make_cfg(n_nodes=100000, d_in=256, d_hid=256, d_out=64, shard_tiles=98,
             n_groups=4, sup=512):
    shard = shard_tiles * 128
    npad = N_CORES * shard
    assert npad % n_groups == 0
    gr = npad // n_groups
    assert gr <= 32768
    assert npad % sup == 0
    assert n_nodes <= npad
    return dict(N=n_nodes, NPAD=npad, SHARD=shard, NT=shard_tiles,
                NG=n_groups, GR=gr, D_IN=d_in, D_HID=d_hid, D_OUT=d_out,
                SUP=sup)


FULL_CFG = make_cfg()


# --------------------------------------------------------------------------
# Host preprocessing
# --------------------------------------------------------------------------

def preprocess(x, edge_index, W1, b1, W2, b2, cfg):
    N, NPAD, SHARD, NT, NG, GR = (cfg["N"], cfg["NPAD"], cfg["SHARD"],
                                  cfg["NT"], cfg["NG"], cfg["GR"])
    D_IN, D_HID, D_OUT = cfg["D_IN"], cfg["D_HID"], cfg["D_OUT"]

    x = np.asarray(x, np.float32)
    edge_index = np.asarray(edge_index)
    src = edge_index[0].astype(np.int64)
    dst = edge_index[1].astype(np.int64)

    deg = np.bincount(dst, minlength=N).astype(np.float32) + 1.0
    dinv = 1.0 / np.sqrt(deg)
    dinv_pad = np.zeros(NPAD, np.float32)
    dinv_pad[:N] = dinv

    # self loops appended as regular edges
    loops = np.arange(N, dtype=np.int64)
    src_all = np.concatenate([src, loops])
    dst_all = np.concatenate([dst, loops])
    E = src_all.shape[0]

    c_of = dst_all // SHARD
    t_of = (dst_all % SHARD) // 128
    d_of = (dst_all % 128).astype(np.float32)
    g_of = src_all // GR
    srcg = (src_all % GR).astype(np.int16)

    key = (c_of * NT + t_of) * NG + g_of
    order = np.argsort(key, kind="stable")
    counts = np.bincount(key, minlength=N_CORES * NT * NG)
    quota = counts.reshape(N_CORES, NT, NG).max(axis=0)
    quota = ((quota + 127) // 128) * 128  # round up to whole 128-slot blocks

    q_flat = quota.reshape(-1)  # [NT*NG]
    call_off = np.zeros(NT * NG + 1, np.int64)
    np.cumsum(q_flat, out=call_off[1:])
    slot_total = int(call_off[-1])

    # slot position of each edge inside its core's stream
    csum = np.zeros(N_CORES * NT * NG + 1, np.int64)
    np.cumsum(counts, out=csum[1:])
    sorted_key = key[order]
    rank = np.arange(E, dtype=np.int64) - csum[sorted_key]
    tg = t_of[order] * NG + g_of[order]
    slot = call_off[tg] + rank
    core = c_of[order]

    idx_arr = np.zeros((N_CORES, slot_total), np.int16)  # pad -> row 0
    dl_arr = np.full((N_CORES, slot_total), PAD_DSTLOC, np.float32)
    idx_arr[core, slot] = srcg[order]
    dl_arr[core, slot] = d_of[order]

    # per-call wrapping: idx wrapped [16, Q/16] (replicated to 128 parts),
    # dstloc wrapped [128, Q/128]
    idxcols = slot_total // 16
    nb = slot_total // 128
    idx_sb = np.zeros((N_CORES, 16, idxcols), np.int16)
    dl_sb = np.zeros((N_CORES, 128, nb), np.float32)
    for tgi in range(NT * NG):
        q = int(q_flat[tgi])
        if q == 0:
            continue
        o = int(call_off[tgi])
        seg = idx_arr[:, o:o + q].reshape(N_CORES, q // 16, 16)
        idx_sb[:, :, o // 16:(o + q) // 16] = seg.transpose(0, 2, 1)
        dseg = dl_arr[:, o:o + q].reshape(N_CORES, q // 128, 128)
        dl_sb[:, :, o // 128:(o + q) // 128] = dseg.transpose(0, 2, 1)
    idx_sb = np.tile(idx_sb, (1, 8, 1))  # replicate to 128 partitions

    # dense inputs
    xp = np.zeros((NPAD, D_IN), np.float32)
    xp[:N] = x
    xT = np.ascontiguousarray(xp.T)  # [D_IN, NPAD]

    ntile = NPAD // 128
    dinv_nodes = np.ascontiguousarray(
        dinv_pad.reshape(ntile, 128).T)  # [128, ntile]
    dinv_dst = np.stack([dinv_nodes[:, c * NT:(c + 1) * NT]
                         for c in range(N_CORES)])  # [8, 128, NT]

    iota = np.tile(np.arange(128, dtype=np.float32), (128, 1))
    ident = np.eye(128, dtype=np.float32)
    b1bc = np.ascontiguousarray(
        np.broadcast_to(np.asarray(b1, np.float32), (128, D_HID)))
    b2bc = np.ascontiguousarray(
        np.broadcast_to(np.asarray(b2, np.float32), (128, D_OUT)))

    common = dict(xT=xT, W1=np.asarray(W1, np.float32),
                  W2=np.asarray(W2, np.float32), b1bc=b1bc, b2bc=b2bc,
                  iota=iota, ident=ident, dinv_nodes=dinv_nodes)
    in_maps = []
    for c in range(N_CORES):
        m = dict(common)
        m["dinv_dst"] = np.ascontiguousarray(dinv_dst[c])
        m["idx_sb"] = np.ascontiguousarray(idx_sb[c])
        m["dstloc"] = np.ascontiguousarray(dl_sb[c])
        in_maps.append(m)

    meta = dict(quota=quota, idxcols=idxcols, nb=nb)
    return in_maps, meta


# --------------------------------------------------------------------------
# Device program
# --------------------------------------------------------------------------

def build_program(cfg, meta, with_collective=True, phases=(1, 2, 3)):
    NPAD, NT, NG, GR, SUP = (cfg["NPAD"], cfg["NT"], cfg["NG"], cfg["GR"],
                             cfg["SUP"])
    D_IN, D_HID, D_OUT = cfg["D_IN"], cfg["D_HID"], cfg["D_OUT"]
    SHARD = cfg["SHARD"]
    quota = meta["quota"]
    idxcols, nb = meta["idxcols"], meta["nb"]
    ntile = NPAD // 128
    KI = D_IN // 128   # k-chunks for layer-1 matmul
    KH = D_HID // 128  # k-chunks for W2 matmul
    cmax1 = min(max(1, int(quota.max()) // 128), QMAX // 128)

    nc = bacc.Bacc("TRN2", target_bir_lowering=False, debug=False,
                   num_devices=N_CORES)

    xT_d = nc.dram_tensor("xT", [D_IN, NPAD], F32, kind="ExternalInput")
    W1_d = nc.dram_tensor("W1", [D_IN, D_HID], F32, kind="ExternalInput")
    W2_d = nc.dram_tensor("W2", [D_HID, D_OUT], F32, kind="ExternalInput")
    b1_d = nc.dram_tensor("b1bc", [128, D_HID], F32, kind="ExternalInput")
    b2_d = nc.dram_tensor("b2bc", [128, D_OUT], F32, kind="ExternalInput")
    iota_d = nc.dram_tensor("iota", [128, 128], F32, kind="ExternalInput")
    ident_d = nc.dram_tensor("ident", [128, 128], F32, kind="ExternalInput")
    dinvn_d = nc.dram_tensor("dinv_nodes", [128, ntile], F32,
                             kind="ExternalInput")
    dinvd_d = nc.dram_tensor("dinv_dst", [128, NT], F32, kind="ExternalInput")
    idx_d = nc.dram_tensor("idx_sb", [128, idxcols], I16, kind="ExternalInput")
    dl_d = nc.dram_tensor("dstloc", [128, nb], F32, kind="ExternalInput")
    out_d = nc.dram_tensor("out", [SHARD, D_OUT], F32, kind="ExternalOutput")

    with tile.TileContext(nc) as tc:
        with (
            tc.tile_pool(name="const", bufs=1) as const,
            tc.tile_pool(name="dram", bufs=1, space="DRAM") as dram,
        ):
            h1p = dram.tile([NPAD, D_HID], F32)
            h2own = dram.tile([SHARD, D_OUT], F32)
            h2full = dram.tile([NPAD, D_OUT], F32, addr_space="Shared")

            w1_sb = const.tile([128, KI, D_HID], F32)
            for k in range(KI):
                nc.sync.dma_start(out=w1_sb[:, k, :],
                                  in_=W1_d.ap()[k * 128:(k + 1) * 128, :])
            w2_sb = const.tile([128, KH, D_OUT], F32)
            for k in range(KH):
                nc.sync.dma_start(out=w2_sb[:, k, :],
                                  in_=W2_d.ap()[k * 128:(k + 1) * 128, :])
            iota_sb = const.tile([128, 128], F32)
            nc.sync.dma_start(out=iota_sb[:], in_=iota_d.ap())
            ident_sb = const.tile([128, 128], F32)
            nc.sync.dma_start(out=ident_sb[:], in_=ident_d.ap())
            b1_sb = const.tile([128, D_HID], F32)
            nc.sync.dma_start(out=b1_sb[:], in_=b1_d.ap())
            b2_sb = const.tile([128, D_OUT], F32)
            nc.sync.dma_start(out=b2_sb[:], in_=b2_d.ap())
            dinvn_sb = const.tile([128, ntile], F32)
            nc.sync.dma_start(out=dinvn_sb[:], in_=dinvn_d.ap())
            dinvd_sb = const.tile([128, NT], F32)
            nc.sync.dma_start(out=dinvd_sb[:], in_=dinvd_d.ap())
            idx_sb = const.tile([128, idxcols], I16)
            nc.sync.dma_start(out=idx_sb[:], in_=idx_d.ap())
            dl_sb = const.tile([128, nb], F32)
            nc.sync.dma_start(out=dl_sb[:], in_=dl_d.ap())

            # ---------------- phase 1: h1p = dinv * (x @ W1) --------------
            nsup = NPAD // SUP if 1 in phases else 0
            nsub = SUP // 128
            xT_r = xT_d.ap().rearrange("(k p) n -> p k n", p=128)
            h1p_r = h1p.rearrange("(s u p) f -> s p u f", p=128, u=nsub)
            with (
                tc.tile_pool(name="p1x", bufs=3) as p1x,
                tc.tile_pool(name="p1o", bufs=3) as p1o,
                tc.tile_pool(name="p1ps", bufs=4, space="PSUM") as p1ps,
            ):
                for s in range(nsup):
                    xt = p1x.tile([128, KI, SUP], F32, tag="xt")
                    nc.sync.dma_start(
                        out=xt[:], in_=xT_r[:, :, s * SUP:(s + 1) * SUP])
                    ot = p1o.tile([128, nsub, D_HID], F32, tag="h1o")
                    for u in range(nsub):
                        ps = p1ps.tile([128, D_HID], F32, tag="p1")
                        for k in range(KI):
                            nc.tensor.matmul(
                                ps[:], xt[:, k, u * 128:(u + 1) * 128],
                                w1_sb[:, k, :],
                                start=(k == 0), stop=(k == KI - 1))
                        gt = s * nsub + u
                        nc.vector.tensor_scalar(
                            ot[:, u, :], ps[:], dinvn_sb[:, gt:gt + 1], None,
                            ALU.mult)
                    nc.sync.dma_start(out=h1p_r[s], in_=ot[:])

            # ---------------- phase 2: layer-1 agg + h2p shard ------------
            h2own_r = h2own.rearrange("(t p) f -> t p f", p=128)

            def agg_phase(table, elem, cmax, epilogue, mtag, stag, ptag,
                          psum_shape, mode="full", dbg_out=None):
                """Shared gather+S+matmul accumulation skeleton.

                mode: "full" | "gather" (skip S/matmul/epilogue, keep a tiny
                consumer per gather) | "agg" (skip real epilogue).
                """
                icol = 0
                blk = 0
                with (
                    tc.tile_pool(name=mtag, bufs=3) as mpool,
                    tc.tile_pool(name=stag, bufs=6) as spool,
                    tc.tile_pool(name=ptag, bufs=2, space="PSUM") as apsum,
                    tc.tile_pool(name=ptag + "ep", bufs=3) as eppool,
                    tc.tile_pool(name=ptag + "ep2", bufs=2,
                                 space="PSUM") as eppsum,
                ):
                    for t in range(NT):
                        nblk_t = int(quota[t].sum()) // 128
                        if nblk_t == 0:
                            continue
                        if mode != "gather":
                            ps = apsum.tile(psum_shape, F32, tag="agg",
                                            name="aggps")
                        else:
                            ps = None
                        bi = 0
                        for g, q0, qc in _chunks(quota[t], NG):
                            q = qc
                            ncols = q // 128
                            mt = mpool.tile([128, cmax, elem], F32, tag="m")
                            nc.gpsimd.dma_gather(
                                mt[:, :ncols, :],
                                table[g * GR:(g + 1) * GR, :],
                                idx_sb[:, icol:icol + q // 16],
                                q, q, elem)
                            icol += q // 16
                            if mode == "gather":
                                nc.sync.dma_start(
                                    out=dbg_out[t * NG + g:t * NG + g + 1, :],
                                    in_=mt[:1, :1, :4])
                                continue
                            for j in range(ncols):
                                st = spool.tile([128, 128], F32, tag="s")
                                nc.vector.tensor_scalar(
                                    st[:], iota_sb[:], dl_sb[:, blk:blk + 1],
                                    None, ALU.is_equal)
                                blk += 1
                                nc.tensor.matmul(
                                    ps[:], st[:], mt[:, j, :],
                                    start=(bi == 0), stop=(bi == nblk_t - 1))
                                bi += 1
                        if mode == "full":
                            epilogue(t, ps, eppool, eppsum)
                        elif mode == "agg":
                            tmp = eppool.tile([128, 4], F32, tag="dbg")
                            nc.vector.tensor_copy(tmp[:], ps[:, :4])
                            nc.sync.dma_start(
                                out=dbg_out[t:t + 1, :], in_=tmp[:1, :])

            def epi1(t, ps, eppool, eppsum):
                o1 = eppool.tile([128, D_HID], F32, tag="o1")
                nc.vector.tensor_scalar(o1[:], ps[:], dinvd_sb[:, t:t + 1],
                                        None, ALU.mult)
                nc.vector.tensor_tensor(o1[:], o1[:], b1_sb[:], ALU.add)
                nc.scalar.activation(o1[:], o1[:], AF.Relu)
                tsb = eppool.tile([128, KH, 128], F32, tag="tsb")
                for k in range(KH):
                    tp = eppsum.tile([128, 128], F32, tag="tr")
                    nc.tensor.transpose(tp[:], o1[:, k * 128:(k + 1) * 128],
                                        ident_sb[:])
                    nc.vector.tensor_copy(tsb[:, k, :], tp[:])
                h2ps = eppsum.tile([128, D_OUT], F32, tag="h2")
                for k in range(KH):
                    nc.tensor.matmul(h2ps[:], tsb[:, k, :], w2_sb[:, k, :],
                                     start=(k == 0), stop=(k == KH - 1))
                h2sb = eppool.tile([128, D_OUT], F32, tag="h2sb")
                nc.vector.tensor_scalar(h2sb[:], h2ps[:],
                                        dinvd_sb[:, t:t + 1], None, ALU.mult)
                nc.sync.dma_start(out=h2own_r[t], in_=h2sb[:])

            import os
            p2_mode = os.environ.get("GCN_P2_MODE", "full")
            dbg = None
            if p2_mode != "full":
                dbg = dram.tile([NT * NG, 4], F32)
            if 2 in phases:
                agg_phase(h1p, D_HID, cmax1, epi1, "m1", "s1", "ag1",
                          [128, D_HID], mode=p2_mode, dbg_out=dbg)

            # ---------------- AllGather h2 shards -------------------------
            if with_collective and 2 in phases:
                nc.gpsimd.collective_compute(
                    "AllGather", ALU.bypass,
                    replica_groups=[list(range(N_CORES))],
                    ins=[h2own.opt()], outs=[h2full.opt()])

            # ---------------- phase 3: layer-2 agg + log_softmax ----------
            out_r = out_d.ap().rearrange("(t p) f -> t p f", p=128)

            def epi2(t, ps, eppool, eppsum):
                t0 = eppool.tile([128, D_OUT], F32, tag="t0")
                nc.vector.tensor_scalar(t0[:], ps[:], dinvd_sb[:, t:t + 1],
                                        None, ALU.mult)
                nc.vector.tensor_tensor(t0[:], t0[:], b2_sb[:], ALU.add)
                nm = eppool.tile([128, 1], F32, tag="nm")
                nc.vector.tensor_reduce(nm[:], t0[:], mybir.AxisListType.X,
                                        ALU.max, negate=True)
                et = eppool.tile([128, D_OUT], F32, tag="et")
                se = eppool.tile([128, 1], F32, tag="se")
                nc.scalar.activation(et[:], t0[:], AF.Exp, bias=nm[:],
                                     accum_out=se[:])
                ls = eppool.tile([128, 1], F32, tag="ls")
                nc.scalar.activation(ls[:], se[:], AF.Ln)
                ot = eppool.tile([128, D_OUT], F32, tag="ot")
                nc.vector.tensor_scalar(ot[:], t0[:], nm[:], ls[:],
                                        ALU.add, ALU.subtract)
                nc.sync.dma_start(out=out_r[t], in_=ot[:])

            if 3 in phases:
                agg_phase(h2full, D_OUT, cmax1, epi2, "m2", "s2", "ag2",
                          [128, D_OUT])

    nc.compile()
    return nc


# --------------------------------------------------------------------------
# Entry point
# --------------------------------------------------------------------------

def kernel(x, edge_index, W1, b1, W2, b2):
    cfg = FULL_CFG
    in_maps, meta = preprocess(x, edge_index, W1, b1, W2, b2, cfg)
    nc = build_program(cfg, meta)
    res = run_bass_kernel_spmd(nc, in_maps, core_ids=list(range(N_CORES)))
    shards = [res.results[c]["out"] for c in range(N_CORES)]
    full = np.concatenate(shards, axis=0)
    return full[:cfg["N"]].astype(np.float32)


# revision 15
# speedup vs baseline: 1.4035x; 1.4035x over previous
"""BasicGCN (2-layer GCN, 100K nodes / 3.2M edges) on 8 Trainium2 NeuronCores.

Strategy (node/dst sharding, graph-parallel):
  - Pad nodes to NPAD = 100352 = 8 * 12544; core c owns dst rows
    [c*12544, (c+1)*12544).
  - Host preprocessing (index-space only): degrees/dinv, per-core edge
    streams sorted by (dst-tile, src-group), self-loops appended as real
    edges, per-(tile,group) slot quotas equalized across cores so one
    SPMD program serves all 8 cores.
  - Device per core:
      phase 1 (dense, replicated): h1p = dinv * (x @ W1) for ALL nodes
        (fp32r matmuls), written to an HBM table [NPAD, 256] bf16.
      phase 2 (layer-1 aggregation): per 128-dst tile, dma_gather h1p
        rows (512B each), build one-hot S blocks on DVE (bf16,
        4 blocks per tensor_tensor via a stride-0 broadcast), segment-sum
        via PE bf16 matmuls into f32 PSUM; epilogue computes
        h2p_own = dinv_d * (relu(dinv_d*agg + b1) @ W2)  -> bf16 shard
        table [12544, 128] (64 data + 64 zero pad).
      AllGather shards -> h2p_full [NPAD, 128] bf16 (Shared DRAM).
      phase 3 (layer-2 aggregation): same gather/S/matmul with 256B rows,
        epilogue log_softmax (f32) -> out shard [12544, 64].
  - Host: concatenate 8 shards, trim to [100000, 64].

Gather tables are split into 4 row-groups of NPAD/4 = 25088 rows so the
int16 gather indices stay in range; each gather call is capped at
QMAX=1024 indices (the q7 dma_gather firmware breaks above ~1024).
"""

import numpy as np

import concourse.bacc as bacc
import concourse.bass as bass
import concourse.mybir as mybir
import concourse.tile as tile
from concourse.bass_utils import run_bass_kernel_spmd

F32 = mybir.dt.float32
F32R = mybir.dt.float32r
BF16 = mybir.dt.bfloat16
I16 = mybir.dt.int16
NP_BF16 = mybir.dt.np(BF16)
AF = mybir.ActivationFunctionType
ALU = mybir.AluOpType

N_CORES = 8
PAD_DSTLOC = 1000.0  # sentinel dst-local for padding slots -> zero S column
QMAX = 1024  # max num_idxs per dma_gather call (HW limit is in (1024, 1280])


def _chunks(quota_row, ng):
    """Split each group's quota into <=QMAX chunks of whole 128-blocks."""
    out = []
    for g in range(ng):
        q = int(quota_row[g])
        off = 0
        while off < q:
            qc = min(QMAX, q - off)
            out.append((g, off, qc))
            off += qc
    return out


def _bcast_last(ap, n):
    """Append a stride-0 broadcast dim of size n to an AP."""
    return bass.AP(ap.tensor, ap.offset, list(ap.ap) + [[0, n]])


def make_cfg(n_nodes=100000, d_in=256, d_hid=256, d_out=64, shard_tiles=98,
             n_groups=4, sup=512):
    shard = shard_tiles * 128
    npad = N_CORES * shard
    assert npad % n_groups == 0
    gr = npad // n_groups
    assert gr <= 32768
    assert npad % sup == 0
    assert n_nodes <= npad
    return dict(N=n_nodes, NPAD=npad, SHARD=shard, NT=shard_tiles,
                NG=n_groups, GR=gr, D_IN=d_in, D_HID=d_hid, D_OUT=d_out,
                SUP=sup)


FULL_CFG = make_cfg()


# --------------------------------------------------------------------------
# Host preprocessing
# --------------------------------------------------------------------------

def preprocess(x, edge_index, W1, b1, W2, b2, cfg):
    N, NPAD, SHARD, NT, NG, GR = (cfg["N"], cfg["NPAD"], cfg["SHARD"],
                                  cfg["NT"], cfg["NG"], cfg["GR"])
    D_IN, D_HID, D_OUT = cfg["D_IN"], cfg["D_HID"], cfg["D_OUT"]

    x = np.asarray(x, np.float32)
    edge_index = np.asarray(edge_index)
    src = edge_index[0].astype(np.int64)
    dst = edge_index[1].astype(np.int64)

    deg = np.bincount(dst, minlength=N).astype(np.float32) + 1.0
    dinv = 1.0 / np.sqrt(deg)
    dinv_pad = np.zeros(NPAD, np.float32)
    dinv_pad[:N] = dinv

    # self loops appended as regular edges
    loops = np.arange(N, dtype=np.int64)
    src_all = np.concatenate([src, loops])
    dst_all = np.concatenate([dst, loops])
    E = src_all.shape[0]

    c_of = dst_all // SHARD
    t_of = (dst_all % SHARD) // 128
    d_of = (dst_all % 128).astype(np.float32)
    g_of = src_all // GR
    srcg = (src_all % GR).astype(np.int16)

    key = (c_of * NT + t_of) * NG + g_of
    order = np.argsort(key, kind="stable")
    counts = np.bincount(key, minlength=N_CORES * NT * NG)
    quota = counts.reshape(N_CORES, NT, NG).max(axis=0)
    quota = ((quota + 127) // 128) * 128  # round up to whole 128-slot blocks

    q_flat = quota.reshape(-1)  # [NT*NG]
    call_off = np.zeros(NT * NG + 1, np.int64)
    np.cumsum(q_flat, out=call_off[1:])
    slot_total = int(call_off[-1])

    # slot position of each edge inside its core's stream
    csum = np.zeros(N_CORES * NT * NG + 1, np.int64)
    np.cumsum(counts, out=csum[1:])
    sorted_key = key[order]
    rank = np.arange(E, dtype=np.int64) - csum[sorted_key]
    tg = t_of[order] * NG + g_of[order]
    slot = call_off[tg] + rank
    core = c_of[order]

    idx_arr = np.zeros((N_CORES, slot_total), np.int16)  # pad -> row 0
    dl_arr = np.full((N_CORES, slot_total), PAD_DSTLOC, np.float32)
    idx_arr[core, slot] = srcg[order]
    dl_arr[core, slot] = d_of[order]

    # per-call wrapping: idx wrapped [16, Q/16] (replicated to 128 parts),
    # dstloc wrapped [128, Q/128]; dstloc padded to whole groups of 4 blocks
    idxcols = slot_total // 16
    nb = slot_total // 128
    nb_pad = ((nb + 3) // 4) * 4
    idx_sb = np.zeros((N_CORES, 16, idxcols), np.int16)
    dl_sb = np.full((N_CORES, 128, nb_pad), PAD_DSTLOC, np.float32)
    for tgi in range(NT * NG):
        q = int(q_flat[tgi])
        if q == 0:
            continue
        o = int(call_off[tgi])
        seg = idx_arr[:, o:o + q].reshape(N_CORES, q // 16, 16)
        idx_sb[:, :, o // 16:(o + q) // 16] = seg.transpose(0, 2, 1)
        dseg = dl_arr[:, o:o + q].reshape(N_CORES, q // 128, 128)
        dl_sb[:, :, o // 128:(o + q) // 128] = dseg.transpose(0, 2, 1)
    idx_sb = np.tile(idx_sb, (1, 8, 1))  # replicate to 128 partitions

    # dense inputs
    xp = np.zeros((NPAD, D_IN), np.float32)
    xp[:N] = x
    xT = np.ascontiguousarray(xp.T)  # [D_IN, NPAD]

    ntile = NPAD // 128
    dinv_nodes = np.ascontiguousarray(
        dinv_pad.reshape(ntile, 128).T)  # [128, ntile]
    dinv_dst = np.stack([dinv_nodes[:, c * NT:(c + 1) * NT]
                         for c in range(N_CORES)])  # [8, 128, NT]

    iota = np.tile(np.arange(128, dtype=np.float32), (128, 1))
    iota4 = np.tile(iota[:, None, :], (1, 4, 1))  # [128, 4, 128]
    ident = np.eye(128, dtype=np.float32)
    b1bc = np.ascontiguousarray(
        np.broadcast_to(np.asarray(b1, np.float32), (128, D_HID)))
    b2bc = np.ascontiguousarray(
        np.broadcast_to(np.asarray(b2, np.float32), (128, D_OUT)))

    common = dict(xT=xT, W1=np.asarray(W1, np.float32),
                  W2=np.asarray(W2, np.float32), b1bc=b1bc, b2bc=b2bc,
                  iota4=iota4.astype(NP_BF16), ident=ident,
                  dinv_nodes=dinv_nodes)
    in_maps = []
    for c in range(N_CORES):
        m = dict(common)
        m["dinv_dst"] = np.ascontiguousarray(dinv_dst[c])
        m["idx_sb"] = np.ascontiguousarray(idx_sb[c])
        m["dstloc"] = np.ascontiguousarray(dl_sb[c]).astype(NP_BF16)
        in_maps.append(m)

    meta = dict(quota=quota, idxcols=idxcols, nb=nb_pad)
    return in_maps, meta


# --------------------------------------------------------------------------
# Device program
# --------------------------------------------------------------------------

def build_program(cfg, meta, with_collective=True, phases=(1, 2, 3)):
    NPAD, NT, NG, GR, SUP = (cfg["NPAD"], cfg["NT"], cfg["NG"], cfg["GR"],
                             cfg["SUP"])
    D_IN, D_HID, D_OUT = cfg["D_IN"], cfg["D_HID"], cfg["D_OUT"]
    SHARD = cfg["SHARD"]
    quota = meta["quota"]
    idxcols, nb = meta["idxcols"], meta["nb"]
    ntile = NPAD // 128
    KI = D_IN // 128   # k-chunks for layer-1 matmul
    KH = D_HID // 128  # k-chunks for W2 matmul
    cmax1 = min(max(1, int(quota.max()) // 128), QMAX // 128)
    D_L2 = 2 * D_OUT  # layer-2 table row: 64 bf16 data + 64 bf16 zeros

    nc = bacc.Bacc("TRN2", target_bir_lowering=False, debug=False,
                   num_devices=N_CORES)

    xT_d = nc.dram_tensor("xT", [D_IN, NPAD], F32R, kind="ExternalInput")
    W1_d = nc.dram_tensor("W1", [D_IN, D_HID], F32R, kind="ExternalInput")
    W2_d = nc.dram_tensor("W2", [D_HID, D_OUT], F32, kind="ExternalInput")
    b1_d = nc.dram_tensor("b1bc", [128, D_HID], F32, kind="ExternalInput")
    b2_d = nc.dram_tensor("b2bc", [128, D_OUT], F32, kind="ExternalInput")
    iota4_d = nc.dram_tensor("iota4", [128, 4, 128], BF16,
                             kind="ExternalInput")
    ident_d = nc.dram_tensor("ident", [128, 128], F32, kind="ExternalInput")
    dinvn_d = nc.dram_tensor("dinv_nodes", [128, ntile], F32,
                             kind="ExternalInput")
    dinvd_d = nc.dram_tensor("dinv_dst", [128, NT], F32, kind="ExternalInput")
    idx_d = nc.dram_tensor("idx_sb", [128, idxcols], I16, kind="ExternalInput")
    dl_d = nc.dram_tensor("dstloc", [128, nb], BF16, kind="ExternalInput")
    out_d = nc.dram_tensor("out", [SHARD, D_OUT], F32, kind="ExternalOutput")

    with tile.TileContext(nc) as tc:
        with (
            tc.tile_pool(name="const", bufs=1) as const,
            tc.tile_pool(name="dram", bufs=1, space="DRAM") as dram,
        ):
            h1p = dram.tile([NPAD, D_HID], BF16)
            h2own = dram.tile([SHARD, D_L2], BF16)
            h2full = dram.tile([NPAD, D_L2], BF16, addr_space="Shared")

            w1_sb = const.tile([128, KI, D_HID], F32R)
            for k in range(KI):
                nc.sync.dma_start(out=w1_sb[:, k, :],
                                  in_=W1_d.ap()[k * 128:(k + 1) * 128, :])
            w2_sb = const.tile([128, KH, D_OUT], F32)
            for k in range(KH):
                nc.sync.dma_start(out=w2_sb[:, k, :],
                                  in_=W2_d.ap()[k * 128:(k + 1) * 128, :])
            iota4_sb = const.tile([128, 4, 128], BF16)
            nc.sync.dma_start(out=iota4_sb[:], in_=iota4_d.ap())
            ident_sb = const.tile([128, 128], F32)
            nc.sync.dma_start(out=ident_sb[:], in_=ident_d.ap())
            b1_sb = const.tile([128, D_HID], F32)
            nc.sync.dma_start(out=b1_sb[:], in_=b1_d.ap())
            b2_sb = const.tile([128, D_OUT], F32)
            nc.sync.dma_start(out=b2_sb[:], in_=b2_d.ap())
            dinvn_sb = const.tile([128, ntile], F32)
            nc.sync.dma_start(out=dinvn_sb[:], in_=dinvn_d.ap())
            dinvd_sb = const.tile([128, NT], F32)
            nc.sync.dma_start(out=dinvd_sb[:], in_=dinvd_d.ap())
            idx_sb = const.tile([128, idxcols], I16)
            nc.sync.dma_start(out=idx_sb[:], in_=idx_d.ap())
            dl_sb = const.tile([128, nb], BF16)
            nc.sync.dma_start(out=dl_sb[:], in_=dl_d.ap())

            # ---------------- phase 1: h1p = dinv * (x @ W1) --------------
            nsup = NPAD // SUP if 1 in phases else 0
            nsub = SUP // 128
            xT_r = xT_d.ap().rearrange("(k p) n -> p k n", p=128)
            h1p_r = h1p.rearrange("(s u p) f -> s p u f", p=128, u=nsub)
            with (
                tc.tile_pool(name="p1x", bufs=3) as p1x,
                tc.tile_pool(name="p1o", bufs=3) as p1o,
                tc.tile_pool(name="p1ps", bufs=4, space="PSUM") as p1ps,
            ):
                for s in range(nsup):
                    xt = p1x.tile([128, KI, SUP], F32R, tag="xt")
                    nc.sync.dma_start(
                        out=xt[:], in_=xT_r[:, :, s * SUP:(s + 1) * SUP])
                    ot = p1o.tile([128, nsub, D_HID], BF16, tag="h1o")
                    for u in range(nsub):
                        ps = p1ps.tile([128, D_HID], F32, tag="p1")
                        for k in range(KI):
                            nc.tensor.matmul(
                                ps[:],
                                xt[:, k, u * 128:(u + 1) * 128],
                                w1_sb[:, k, :],
                                start=(k == 0), stop=(k == KI - 1))
                        gt = s * nsub + u
                        nc.vector.tensor_scalar(
                            ot[:, u, :], ps[:], dinvn_sb[:, gt:gt + 1], None,
                            ALU.mult)
                    nc.sync.dma_start(out=h1p_r[s], in_=ot[:])

            # ---------------- phase 2: layer-1 agg + h2p shard ------------
            h2own_r = h2own.rearrange("(t p) f -> t p f", p=128)

            def agg_phase(table, elem, rhs_w, cmax, epilogue, mtag, stag,
                          ptag, psum_shape):
                """Gather + one-hot-S + matmul accumulation skeleton.

                S blocks are built 4 at a time: one bf16 tensor_tensor
                compare of iota4 [128,4,128] against dstloc broadcast.
                """
                icol = 0
                blk = 0
                state = {"s4": None, "s4_base": -1}

                def get_s(bi_global, spool):
                    b4 = (bi_global // 4) * 4
                    if state["s4_base"] != b4:
                        s4 = spool.tile([128, 4, 128], BF16, tag="s4",
                                        name="s4t")
                        nc.vector.tensor_tensor(
                            s4[:], iota4_sb[:],
                            _bcast_last(dl_sb[:, b4:b4 + 4], 128),
                            ALU.is_equal)
                        state["s4"] = s4
                        state["s4_base"] = b4
                    return state["s4"][:, bi_global - b4, :]

                with (
                    tc.tile_pool(name=mtag, bufs=3) as mpool,
                    tc.tile_pool(name=stag, bufs=4) as spool,
                    tc.tile_pool(name=ptag, bufs=2, space="PSUM") as apsum,
                    tc.tile_pool(name=ptag + "ep", bufs=3) as eppool,
                    tc.tile_pool(name=ptag + "ep2", bufs=2,
                                 space="PSUM") as eppsum,
                ):
                    for t in range(NT):
                        nblk_t = int(quota[t].sum()) // 128
                        if nblk_t == 0:
                            continue
                        ps = apsum.tile(psum_shape, F32, tag="agg",
                                        name="aggps")
                        bi = 0
                        for g, q0, q in _chunks(quota[t], NG):
                            ncols = q // 128
                            mt = mpool.tile([128, cmax, elem], BF16, tag="m")
                            nc.gpsimd.dma_gather(
                                mt[:, :ncols, :],
                                table[g * GR:(g + 1) * GR, :],
                                idx_sb[:, icol:icol + q // 16],
                                q, q, elem)
                            icol += q // 16
                            for j in range(ncols):
                                st = get_s(blk, spool)
                                blk += 1
                                nc.tensor.matmul(
                                    ps[:], st, mt[:, j, :rhs_w],
                                    start=(bi == 0), stop=(bi == nblk_t - 1))
                                bi += 1
                        epilogue(t, ps, eppool, eppsum)

            def epi1(t, ps, eppool, eppsum):
                o1 = eppool.tile([128, D_HID], F32, tag="o1")
                nc.vector.tensor_scalar(o1[:], ps[:], dinvd_sb[:, t:t + 1],
                                        None, ALU.mult)
                nc.vector.tensor_tensor(o1[:], o1[:], b1_sb[:], ALU.add)
                nc.scalar.activation(o1[:], o1[:], AF.Relu)
                tsb = eppool.tile([128, KH, 128], F32, tag="tsb")
                for k in range(KH):
                    tp = eppsum.tile([128, 128], F32, tag="tr")
                    nc.tensor.transpose(tp[:], o1[:, k * 128:(k + 1) * 128],
                                        ident_sb[:])
                    nc.vector.tensor_copy(tsb[:, k, :], tp[:])
                h2ps = eppsum.tile([128, D_OUT], F32, tag="h2")
                for k in range(KH):
                    nc.tensor.matmul(h2ps[:], tsb[:, k, :], w2_sb[:, k, :],
                                     start=(k == 0), stop=(k == KH - 1))
                h2sb = eppool.tile([128, D_L2], BF16, tag="h2sb")
                nc.vector.tensor_scalar(h2sb[:, :D_OUT], h2ps[:],
                                        dinvd_sb[:, t:t + 1], None, ALU.mult)
                nc.vector.memset(h2sb[:, D_OUT:], 0.0)
                nc.sync.dma_start(out=h2own_r[t], in_=h2sb[:])

            if 2 in phases:
                agg_phase(h1p, D_HID, D_HID, cmax1, epi1, "m1", "s1", "ag1",
                          [128, D_HID])

            # ---------------- AllGather h2 shards -------------------------
            if with_collective and 2 in phases:
                nc.gpsimd.collective_compute(
                    "AllGather", ALU.bypass,
                    replica_groups=[list(range(N_CORES))],
                    ins=[h2own.opt()], outs=[h2full.opt()])

            # ---------------- phase 3: layer-2 agg + log_softmax ----------
            out_r = out_d.ap().rearrange("(t p) f -> t p f", p=128)

            def epi2(t, ps, eppool, eppsum):
                t0 = eppool.tile([128, D_OUT], F32, tag="t0")
                nc.vector.tensor_scalar(t0[:], ps[:], dinvd_sb[:, t:t + 1],
                                        None, ALU.mult)
                nc.vector.tensor_tensor(t0[:], t0[:], b2_sb[:], ALU.add)
                nm = eppool.tile([128, 1], F32, tag="nm")
                nc.vector.tensor_reduce(nm[:], t0[:], mybir.AxisListType.X,
                                        ALU.max, negate=True)
                et = eppool.tile([128, D_OUT], F32, tag="et")
                se = eppool.tile([128, 1], F32, tag="se")
                nc.scalar.activation(et[:], t0[:], AF.Exp, bias=nm[:],
                                     accum_out=se[:])
                ls = eppool.tile([128, 1], F32, tag="ls")
                nc.scalar.activation(ls[:], se[:], AF.Ln)
                ot = eppool.tile([128, D_OUT], F32, tag="ot")
                nc.vector.tensor_scalar(ot[:], t0[:], nm[:], ls[:],
                                        ALU.add, ALU.subtract)
                nc.sync.dma_start(out=out_r[t], in_=ot[:])

            if 3 in phases:
                agg_phase(h2full, D_L2, D_OUT, cmax1, epi2, "m2", "s2",
                          "ag2", [128, D_OUT])

    nc.compile()
    return nc


# --------------------------------------------------------------------------
# Entry point
# --------------------------------------------------------------------------

def kernel(x, edge_index, W1, b1, W2, b2):
    cfg = FULL_CFG
    in_maps, meta = preprocess(x, edge_index, W1, b1, W2, b2, cfg)
    nc = build_program(cfg, meta)
    res = run_bass_kernel_spmd(nc, in_maps, core_ids=list(range(N_CORES)))
    shards = [res.results[c]["out"] for c in range(N_CORES)]
    full = np.concatenate(shards, axis=0)
    return full[:cfg["N"]].astype(np.float32)


# revision 18
# speedup vs baseline: 2.1755x; 1.5501x over previous
"""BasicGCN (2-layer GCN, 100K nodes / 3.2M edges) on 8 Trainium2 NeuronCores.

Strategy (node/dst sharding, graph-parallel):
  - Pad nodes to NPAD = 100352 = 8 * 12544; core c owns dst rows
    [c*12544, (c+1)*12544).
  - Host preprocessing (index-space only): degrees/dinv, per-core edge
    streams bucketed by (dst-tile-quad, src-group, dst-tile), self-loops
    appended as real edges, per-(tile,group) slot quotas equalized across
    cores so one SPMD program serves all 8 cores.
  - Device per core:
      phase 1 (dense, replicated): h1p = dinv * (x @ W1) for ALL nodes
        (fp32r matmuls), written to an HBM table [NPAD, 256] bf16.
      phase 2 (layer-1 aggregation): dma_gather h1p rows (512B each) in
        <=1024-row calls spanning a quad of dst tiles, build one-hot S
        blocks on DVE (S[e,d] = dst_local[e]==d), segment-sum via PE bf16
        matmuls into per-tile f32 PSUM accumulators; epilogue computes
        h2p_own = dinv_d * (relu(dinv_d*agg + b1) @ W2)  -> bf16 shard
        table [12544, 128] (64 data + 64 zero pad).
      AllGather shards -> h2p_full [NPAD, 128] bf16 (Shared DRAM).
      phase 3 (layer-2 aggregation): same gather/S/matmul with 256B rows,
        epilogue log_softmax (f32) -> out shard [12544, 64].
  - Host: concatenate 8 shards, trim to [100000, 64].

Gather tables are split into 4 row-groups of NPAD/4 = 25088 rows so the
int16 gather indices stay in range; each dma_gather call is capped at
QMAX=1024 indices (the q7 firmware breaks above ~1024) and spans the
quad's whole per-group run to keep calls full (SWDGE fixed cost is the
main Pool-engine expense).
"""

import numpy as np

import concourse.bacc as bacc
import concourse.bass as bass
import concourse.mybir as mybir
import concourse.tile as tile
from concourse.bass_utils import run_bass_kernel_spmd

F32 = mybir.dt.float32
F32R = mybir.dt.float32r
BF16 = mybir.dt.bfloat16
I16 = mybir.dt.int16
NP_BF16 = mybir.dt.np(BF16)
AF = mybir.ActivationFunctionType
ALU = mybir.AluOpType

N_CORES = 8
PAD_DSTLOC = 1000.0  # sentinel dst-local for padding slots -> zero S column
QMAX = 1024  # max num_idxs per dma_gather call (HW limit is in (1024, 1280])
QT = 4       # dst tiles per gather bucket (quad)


def make_cfg(n_nodes=100000, d_in=256, d_hid=256, d_out=64, shard_tiles=98,
             n_groups=4, sup=512):
    shard = shard_tiles * 128
    npad = N_CORES * shard
    assert npad % n_groups == 0
    gr = npad // n_groups
    assert gr <= 32768
    assert npad % sup == 0
    assert n_nodes <= npad
    return dict(N=n_nodes, NPAD=npad, SHARD=shard, NT=shard_tiles,
                NG=n_groups, GR=gr, D_IN=d_in, D_HID=d_hid, D_OUT=d_out,
                SUP=sup)


FULL_CFG = make_cfg()


def _build_schedule(quota, nt, ng):
    """Gather-call schedule over (quad, group) runs.

    Returns (calls, blk_tile, call_off_flat, slot_total):
      calls: list of (g, icol, q) in stream order, q <= QMAX, all %128==0
      blk_tile: tile id per 128-slot block, in stream order
      call_off_flat[t*ng+g]: slot offset of the (t,g) section
    """
    call_off_flat = np.zeros(nt * ng, np.int64)
    blk_tile = []
    calls = []
    off = 0
    for qd in range(0, nt, QT):
        tiles = range(qd, min(qd + QT, nt))
        for g in range(ng):
            total = 0
            for t in tiles:
                q = int(quota[t, g])
                call_off_flat[t * ng + g] = off + total
                blk_tile.extend([t] * (q // 128))
                total += q
            if total == 0:
                continue
            nblk = total // 128
            nch = (total + QMAX - 1) // QMAX
            base, rem = divmod(nblk, nch)
            o = off
            for i in range(nch):
                q = (base + (1 if i < rem else 0)) * 128
                calls.append((g, o, q))
                o += q
            off += total
    return calls, blk_tile, call_off_flat, off


# --------------------------------------------------------------------------
# Host preprocessing
# --------------------------------------------------------------------------

def preprocess(x, edge_index, W1, b1, W2, b2, cfg):
    N, NPAD, SHARD, NT, NG, GR = (cfg["N"], cfg["NPAD"], cfg["SHARD"],
                                  cfg["NT"], cfg["NG"], cfg["GR"])
    D_IN, D_HID, D_OUT = cfg["D_IN"], cfg["D_HID"], cfg["D_OUT"]

    x = np.asarray(x, np.float32)
    edge_index = np.asarray(edge_index)
    src = edge_index[0].astype(np.int64)
    dst = edge_index[1].astype(np.int64)

    deg = np.bincount(dst, minlength=N).astype(np.float32) + 1.0
    dinv = 1.0 / np.sqrt(deg)
    dinv_pad = np.zeros(NPAD, np.float32)
    dinv_pad[:N] = dinv

    # self loops appended as regular edges
    loops = np.arange(N, dtype=np.int64)
    src_all = np.concatenate([src, loops])
    dst_all = np.concatenate([dst, loops])
    E = src_all.shape[0]

    c_of = dst_all // SHARD
    t_of = (dst_all % SHARD) // 128
    d_of = (dst_all % 128).astype(np.float32)
    g_of = src_all // GR
    srcg = (src_all % GR).astype(np.int16)

    key = (c_of * NT + t_of) * NG + g_of
    order = np.argsort(key, kind="stable")
    counts = np.bincount(key, minlength=N_CORES * NT * NG)
    quota = counts.reshape(N_CORES, NT, NG).max(axis=0)
    quota = ((quota + 127) // 128) * 128  # round up to whole 128-slot blocks

    calls, blk_tile, call_off_flat, slot_total = _build_schedule(
        quota, NT, NG)

    # slot position of each edge inside its core's stream
    csum = np.zeros(N_CORES * NT * NG + 1, np.int64)
    np.cumsum(counts, out=csum[1:])
    sorted_key = key[order]
    rank = np.arange(E, dtype=np.int64) - csum[sorted_key]
    tg = t_of[order] * NG + g_of[order]
    slot = call_off_flat[tg] + rank
    core = c_of[order]

    idx_arr = np.zeros((N_CORES, slot_total), np.int16)  # pad -> row 0
    dl_arr = np.full((N_CORES, slot_total), PAD_DSTLOC, np.float32)
    idx_arr[core, slot] = srcg[order]
    dl_arr[core, slot] = d_of[order]

    # per-section wrapping: idx wrapped [16, Q/16] (replicated to 128
    # parts), dstloc wrapped [128, Q/128]
    idxcols = slot_total // 16
    nb = slot_total // 128
    idx_sb = np.zeros((N_CORES, 16, idxcols), np.int16)
    dl_sb = np.full((N_CORES, 128, nb), PAD_DSTLOC, np.float32)
    for tgi in range(NT * NG):
        q = int(quota.reshape(-1)[tgi])
        if q == 0:
            continue
        o = int(call_off_flat[tgi])
        seg = idx_arr[:, o:o + q].reshape(N_CORES, q // 16, 16)
        idx_sb[:, :, o // 16:(o + q) // 16] = seg.transpose(0, 2, 1)
        dseg = dl_arr[:, o:o + q].reshape(N_CORES, q // 128, 128)
        dl_sb[:, :, o // 128:(o + q) // 128] = dseg.transpose(0, 2, 1)
    idx_sb = np.tile(idx_sb, (1, 8, 1))  # replicate to 128 partitions

    # dense inputs
    xp = np.zeros((NPAD, D_IN), np.float32)
    xp[:N] = x
    xT = np.ascontiguousarray(xp.T)  # [D_IN, NPAD]

    ntile = NPAD // 128
    dinv_nodes = np.ascontiguousarray(
        dinv_pad.reshape(ntile, 128).T)  # [128, ntile]
    dinv_dst = np.stack([dinv_nodes[:, c * NT:(c + 1) * NT]
                         for c in range(N_CORES)])  # [8, 128, NT]

    iota = np.tile(np.arange(128), (128, 1)).astype(NP_BF16)
    ident = np.eye(128, dtype=np.float32)
    b1bc = np.ascontiguousarray(
        np.broadcast_to(np.asarray(b1, np.float32), (128, D_HID)))
    b2bc = np.ascontiguousarray(
        np.broadcast_to(np.asarray(b2, np.float32), (128, D_OUT)))

    common = dict(xT=xT.astype(NP_BF16), W1=np.asarray(W1, NP_BF16),
                  W2=np.asarray(W2, np.float32), b1bc=b1bc, b2bc=b2bc,
                  iota=iota, ident=ident, dinv_nodes=dinv_nodes)
    in_maps = []
    for c in range(N_CORES):
        m = dict(common)
        m["dinv_dst"] = np.ascontiguousarray(dinv_dst[c])
        m["idx_sb"] = np.ascontiguousarray(idx_sb[c])
        m["dstloc"] = np.ascontiguousarray(dl_sb[c])
        in_maps.append(m)

    meta = dict(quota=quota, idxcols=idxcols, nb=nb, calls=calls,
                blk_tile=blk_tile)
    return in_maps, meta


# --------------------------------------------------------------------------
# Device program
# --------------------------------------------------------------------------

def build_program(cfg, meta, with_collective=True, phases=(1, 2, 3)):
    NPAD, NT, NG, GR, SUP = (cfg["NPAD"], cfg["NT"], cfg["NG"], cfg["GR"],
                             cfg["SUP"])
    D_IN, D_HID, D_OUT = cfg["D_IN"], cfg["D_HID"], cfg["D_OUT"]
    SHARD = cfg["SHARD"]
    quota = meta["quota"]
    idxcols, nb = meta["idxcols"], meta["nb"]
    calls, blk_tile = meta["calls"], meta["blk_tile"]
    ntile = NPAD // 128
    KI = D_IN // 128   # k-chunks for layer-1 matmul
    KH = D_HID // 128  # k-chunks for W2 matmul
    CMAX = QMAX // 128
    D_L2 = 2 * D_OUT  # layer-2 table row: 64 bf16 data + 64 bf16 zeros

    # first/last block of each tile (accumulation start/stop flags)
    first_blk = {}
    last_blk = {}
    for i, t in enumerate(blk_tile):
        first_blk.setdefault(t, i)
        last_blk[t] = i

    nc = bacc.Bacc("TRN2", target_bir_lowering=False, debug=False,
                   num_devices=N_CORES)

    xT_d = nc.dram_tensor("xT", [D_IN, NPAD], BF16, kind="ExternalInput")
    W1_d = nc.dram_tensor("W1", [D_IN, D_HID], BF16, kind="ExternalInput")
    W2_d = nc.dram_tensor("W2", [D_HID, D_OUT], F32, kind="ExternalInput")
    b1_d = nc.dram_tensor("b1bc", [128, D_HID], F32, kind="ExternalInput")
    b2_d = nc.dram_tensor("b2bc", [128, D_OUT], F32, kind="ExternalInput")
    iota_d = nc.dram_tensor("iota", [128, 128], BF16, kind="ExternalInput")
    ident_d = nc.dram_tensor("ident", [128, 128], F32, kind="ExternalInput")
    dinvn_d = nc.dram_tensor("dinv_nodes", [128, ntile], F32,
                             kind="ExternalInput")
    dinvd_d = nc.dram_tensor("dinv_dst", [128, NT], F32, kind="ExternalInput")
    idx_d = nc.dram_tensor("idx_sb", [128, idxcols], I16, kind="ExternalInput")
    dl_d = nc.dram_tensor("dstloc", [128, nb], F32, kind="ExternalInput")
    out_d = nc.dram_tensor("out", [SHARD, D_OUT], F32, kind="ExternalOutput")

    with tile.TileContext(nc) as tc:
        with (
            tc.tile_pool(name="const", bufs=1) as const,
            tc.tile_pool(name="dram", bufs=1, space="DRAM") as dram,
        ):
            h1p_g = [dram.tile([GR, D_HID], BF16, name=f"h1p{g}")
                     for g in range(NG)]
            h2own = dram.tile([SHARD, D_L2], BF16)
            h2full = dram.tile([NPAD, D_L2], BF16, addr_space="Shared")

            w1_sb = const.tile([128, KI, D_HID], BF16)
            for k in range(KI):
                nc.sync.dma_start(out=w1_sb[:, k, :],
                                  in_=W1_d.ap()[k * 128:(k + 1) * 128, :])
            w2_sb = const.tile([128, KH, D_OUT], F32)
            for k in range(KH):
                nc.sync.dma_start(out=w2_sb[:, k, :],
                                  in_=W2_d.ap()[k * 128:(k + 1) * 128, :])
            iota_sb = const.tile([128, 128], BF16)
            nc.sync.dma_start(out=iota_sb[:], in_=iota_d.ap())
            ident_sb = const.tile([128, 128], F32)
            nc.sync.dma_start(out=ident_sb[:], in_=ident_d.ap())
            b1_sb = const.tile([128, D_HID], F32)
            nc.sync.dma_start(out=b1_sb[:], in_=b1_d.ap())
            b2_sb = const.tile([128, D_OUT], F32)
            nc.sync.dma_start(out=b2_sb[:], in_=b2_d.ap())
            dinvn_sb = const.tile([128, ntile], F32)
            nc.sync.dma_start(out=dinvn_sb[:], in_=dinvn_d.ap())
            dinvd_sb = const.tile([128, NT], F32)
            nc.sync.dma_start(out=dinvd_sb[:], in_=dinvd_d.ap())
            idx_sb = const.tile([128, idxcols], I16)
            nc.sync.dma_start(out=idx_sb[:], in_=idx_d.ap())
            dl_sb = const.tile([128, nb], F32)
            nc.sync.dma_start(out=dl_sb[:], in_=dl_d.ap())

            # ---------------- phase 1: h1p = dinv * (x @ W1) --------------
            nsup = NPAD // SUP if 1 in phases else 0
            nsub = SUP // 128
            assert GR % SUP == 0
            sup_per_g = GR // SUP
            xT_r = xT_d.ap().rearrange("(k p) n -> p k n", p=128)
            h1p_g_r = [t.rearrange("(s u p) f -> s p u f", p=128, u=nsub)
                       for t in h1p_g]
            with (
                tc.tile_pool(name="p1x", bufs=3) as p1x,
                tc.tile_pool(name="p1o", bufs=3) as p1o,
                tc.tile_pool(name="p1ps", bufs=4, space="PSUM") as p1ps,
            ):
                for s in range(nsup):
                    xt = p1x.tile([128, KI, SUP], BF16, tag="xt")
                    nc.sync.dma_start(
                        out=xt[:], in_=xT_r[:, :, s * SUP:(s + 1) * SUP])
                    ot = p1o.tile([128, nsub, D_HID], BF16, tag="h1o")
                    for u in range(nsub):
                        ps = p1ps.tile([128, D_HID], F32, tag="p1")
                        for k in range(KI):
                            nc.tensor.matmul(
                                ps[:],
                                xt[:, k, u * 128:(u + 1) * 128],
                                w1_sb[:, k, :],
                                start=(k == 0), stop=(k == KI - 1))
                        gt = s * nsub + u
                        nc.scalar.activation(
                            ot[:, u, :], ps[:], AF.Copy,
                            scale=dinvn_sb[:, gt:gt + 1])
                    nc.sync.dma_start(
                        out=h1p_g_r[s // sup_per_g][s % sup_per_g],
                        in_=ot[:])

            # ---------------- phase 2: layer-1 agg + h2p shard ------------
            h2own_r = h2own.rearrange("(t p) f -> t p f", p=128)

            def agg_phase(table, elem, rhs_w, epilogue, mtag, stag, ptag):
                """Gather + one-hot-S + matmul accumulation over the
                precomputed quad-spanning call schedule."""
                blk = 0
                psums = {}
                with (
                    tc.tile_pool(name=mtag, bufs=6) as mpool,
                    tc.tile_pool(name=stag, bufs=8) as spool,
                    tc.tile_pool(name=ptag, bufs=5, space="PSUM") as apsum,
                    tc.tile_pool(name=ptag + "ep", bufs=2) as eppool,
                    tc.tile_pool(name=ptag + "ep2", bufs=1,
                                 space="PSUM") as eppsum,
                ):
                    for g, o, q in calls:
                        ncols = q // 128
                        mt = mpool.tile([128, CMAX, elem], BF16, tag="m")
                        nc.gpsimd.dma_gather(
                            mt[:, :ncols, :],
                            table(g),
                            idx_sb[:, o // 16:(o + q) // 16],
                            q, q, elem)
                        for j in range(ncols):
                            t = blk_tile[blk]
                            if blk == first_blk[t]:
                                psums[t] = apsum.tile(
                                    [128, rhs_w], F32, tag="agg",
                                    name="aggps")
                            st = spool.tile([128, 128], BF16, tag="s",
                                            name="stile")
                            nc.vector.tensor_scalar(
                                st[:], iota_sb[:], dl_sb[:, blk:blk + 1],
                                None, ALU.is_equal)
                            nc.tensor.matmul(
                                psums[t][:], st[:], mt[:, j, :rhs_w],
                                start=(blk == first_blk[t]),
                                stop=(blk == last_blk[t]))
                            if blk == last_blk[t]:
                                epilogue(t, psums.pop(t), eppool, eppsum)
                            blk += 1

            def epi1(t, ps, eppool, eppsum):
                o1 = eppool.tile([128, D_HID], F32, tag="o1")
                nc.vector.tensor_scalar(o1[:], ps[:], dinvd_sb[:, t:t + 1],
                                        None, ALU.mult)
                nc.vector.tensor_tensor(o1[:], o1[:], b1_sb[:], ALU.add)
                nc.scalar.activation(o1[:], o1[:], AF.Relu)
                tsb = eppool.tile([128, KH, 128], F32, tag="tsb")
                for k in range(KH):
                    tp = eppsum.tile([128, 128], F32, tag="tr")
                    nc.tensor.transpose(tp[:], o1[:, k * 128:(k + 1) * 128],
                                        ident_sb[:])
                    nc.vector.tensor_copy(tsb[:, k, :], tp[:])
                h2ps = eppsum.tile([128, D_OUT], F32, tag="h2")
                for k in range(KH):
                    nc.tensor.matmul(h2ps[:], tsb[:, k, :], w2_sb[:, k, :],
                                     start=(k == 0), stop=(k == KH - 1))
                h2sb = eppool.tile([128, D_L2], BF16, tag="h2sb")
                nc.vector.tensor_scalar(h2sb[:, :D_OUT], h2ps[:],
                                        dinvd_sb[:, t:t + 1], None, ALU.mult)
                nc.vector.memset(h2sb[:, D_OUT:], 0.0)
                nc.sync.dma_start(out=h2own_r[t], in_=h2sb[:])

            if 2 in phases:
                agg_phase(lambda g: h1p_g[g][:, :], D_HID, D_HID, epi1,
                          "m1", "s1", "ag1")

            # ---------------- AllGather h2 shards -------------------------
            if with_collective and 2 in phases:
                nc.gpsimd.collective_compute(
                    "AllGather", ALU.bypass,
                    replica_groups=[list(range(N_CORES))],
                    ins=[h2own.opt()], outs=[h2full.opt()])

            # ---------------- phase 3: layer-2 agg + log_softmax ----------
            out_r = out_d.ap().rearrange("(t p) f -> t p f", p=128)

            def epi2(t, ps, eppool, eppsum):
                t0 = eppool.tile([128, D_OUT], F32, tag="t0")
                nc.vector.tensor_scalar(t0[:], ps[:], dinvd_sb[:, t:t + 1],
                                        None, ALU.mult)
                nc.vector.tensor_tensor(t0[:], t0[:], b2_sb[:], ALU.add)
                nm = eppool.tile([128, 1], F32, tag="nm")
                nc.vector.tensor_reduce(nm[:], t0[:], mybir.AxisListType.X,
                                        ALU.max, negate=True)
                et = eppool.tile([128, D_OUT], F32, tag="et")
                se = eppool.tile([128, 1], F32, tag="se")
                nc.scalar.activation(et[:], t0[:], AF.Exp, bias=nm[:],
                                     accum_out=se[:])
                ls = eppool.tile([128, 1], F32, tag="ls")
                nc.scalar.activation(ls[:], se[:], AF.Ln)
                ot = eppool.tile([128, D_OUT], F32, tag="ot")
                nc.vector.tensor_scalar(ot[:], t0[:], nm[:], ls[:],
                                        ALU.add, ALU.subtract)
                nc.sync.dma_start(out=out_r[t], in_=ot[:])

            if 3 in phases:
                agg_phase(lambda g: h2full[g * GR:(g + 1) * GR, :], D_L2,
                          D_OUT, epi2, "m2", "s2", "ag2")

    nc.compile()
    return nc


# --------------------------------------------------------------------------
# Entry point
# --------------------------------------------------------------------------

def kernel(x, edge_index, W1, b1, W2, b2):
    cfg = FULL_CFG
    in_maps, meta = preprocess(x, edge_index, W1, b1, W2, b2, cfg)
    nc = build_program(cfg, meta)
    res = run_bass_kernel_spmd(nc, in_maps, core_ids=list(range(N_CORES)))
    shards = [res.results[c]["out"] for c in range(N_CORES)]
    full = np.concatenate(shards, axis=0)
    return full[:cfg["N"]].astype(np.float32)


# revision 20
# speedup vs baseline: 2.2520x; 1.0352x over previous
"""BasicGCN (2-layer GCN, 100K nodes / 3.2M edges) on 8 Trainium2 NeuronCores.

Strategy (node/dst sharding, graph-parallel):
  - Pad nodes to NPAD = 100352 = 8 * 12544; core c owns dst rows
    [c*12544, (c+1)*12544).
  - Host preprocessing (index-space only): degrees/dinv, per-core edge
    streams bucketed by (dst-tile-quad, src-group, dst-tile), self-loops
    appended as real edges, per-(tile,group) slot quotas equalized across
    cores so one SPMD program serves all 8 cores.
  - Device per core:
      phase 1 (dense, replicated): h1p = dinv * (x @ W1) for ALL nodes
        (fp32r matmuls), written to an HBM table [NPAD, 256] bf16.
      phase 2 (layer-1 aggregation): dma_gather h1p rows (512B each) in
        <=1024-row calls spanning a quad of dst tiles, build one-hot S
        blocks on DVE (S[e,d] = dst_local[e]==d), segment-sum via PE bf16
        matmuls into per-tile f32 PSUM accumulators; epilogue computes
        h2p_own = dinv_d * (relu(dinv_d*agg + b1) @ W2)  -> bf16 shard
        table [12544, 128] (64 data + 64 zero pad).
      AllGather shards -> h2p_full [NPAD, 128] bf16 (Shared DRAM).
      phase 3 (layer-2 aggregation): same gather/S/matmul with 256B rows,
        epilogue log_softmax (f32) -> out shard [12544, 64].
  - Host: concatenate 8 shards, trim to [100000, 64].

Gather tables are split into 4 row-groups of NPAD/4 = 25088 rows so the
int16 gather indices stay in range; each dma_gather call is capped at
QMAX=1024 indices (the q7 firmware breaks above ~1024) and spans the
quad's whole per-group run to keep calls full (SWDGE fixed cost is the
main Pool-engine expense).
"""

import numpy as np

import concourse.bacc as bacc
import concourse.bass as bass
import concourse.mybir as mybir
import concourse.tile as tile
from concourse.bass_utils import run_bass_kernel_spmd

F32 = mybir.dt.float32
F32R = mybir.dt.float32r
BF16 = mybir.dt.bfloat16
I16 = mybir.dt.int16
NP_BF16 = mybir.dt.np(BF16)
AF = mybir.ActivationFunctionType
ALU = mybir.AluOpType

N_CORES = 8
PAD_DSTLOC = 1000.0  # sentinel dst-local for padding slots -> zero S column
QMAX = 1024  # max num_idxs per dma_gather call (HW limit is in (1024, 1280])
QT = 4       # dst tiles per gather bucket (quad)


def make_cfg(n_nodes=100000, d_in=256, d_hid=256, d_out=64, shard_tiles=98,
             n_groups=4, sup=512):
    shard = shard_tiles * 128
    npad = N_CORES * shard
    assert npad % n_groups == 0
    gr = npad // n_groups
    assert gr <= 32768
    assert npad % sup == 0
    assert n_nodes <= npad
    return dict(N=n_nodes, NPAD=npad, SHARD=shard, NT=shard_tiles,
                NG=n_groups, GR=gr, D_IN=d_in, D_HID=d_hid, D_OUT=d_out,
                SUP=sup)


FULL_CFG = make_cfg()


def _build_schedule(quota, nt, ng):
    """Gather-call schedule over (quad, group) runs.

    Returns (calls, blk_tile, call_off_flat, slot_total):
      calls: list of (g, icol, q) in stream order, q <= QMAX, all %128==0
      blk_tile: tile id per 128-slot block, in stream order
      call_off_flat[t*ng+g]: slot offset of the (t,g) section
    """
    call_off_flat = np.zeros(nt * ng, np.int64)
    blk_tile = []
    calls = []
    off = 0
    for qd in range(0, nt, QT):
        tiles = range(qd, min(qd + QT, nt))
        for g in range(ng):
            total = 0
            for t in tiles:
                q = int(quota[t, g])
                call_off_flat[t * ng + g] = off + total
                blk_tile.extend([t] * (q // 128))
                total += q
            if total == 0:
                continue
            nblk = total // 128
            nch = (total + QMAX - 1) // QMAX
            base, rem = divmod(nblk, nch)
            o = off
            for i in range(nch):
                q = (base + (1 if i < rem else 0)) * 128
                calls.append((g, o, q))
                o += q
            off += total
    return calls, blk_tile, call_off_flat, off


# --------------------------------------------------------------------------
# Host preprocessing
# --------------------------------------------------------------------------

def preprocess(x, edge_index, W1, b1, W2, b2, cfg):
    N, NPAD, SHARD, NT, NG, GR = (cfg["N"], cfg["NPAD"], cfg["SHARD"],
                                  cfg["NT"], cfg["NG"], cfg["GR"])
    D_IN, D_HID, D_OUT = cfg["D_IN"], cfg["D_HID"], cfg["D_OUT"]

    x = np.asarray(x, np.float32)
    edge_index = np.asarray(edge_index)
    src = edge_index[0].astype(np.int64)
    dst = edge_index[1].astype(np.int64)

    deg = np.bincount(dst, minlength=N).astype(np.float32) + 1.0
    dinv = 1.0 / np.sqrt(deg)
    dinv_pad = np.zeros(NPAD, np.float32)
    dinv_pad[:N] = dinv

    # self loops appended as regular edges
    loops = np.arange(N, dtype=np.int64)
    src_all = np.concatenate([src, loops])
    dst_all = np.concatenate([dst, loops])
    E = src_all.shape[0]

    c_of = dst_all // SHARD
    t_of = (dst_all % SHARD) // 128
    d_of = (dst_all % 128).astype(np.float32)
    g_of = src_all // GR
    srcg = (src_all % GR).astype(np.int16)

    key = (c_of * NT + t_of) * NG + g_of
    order = np.argsort(key, kind="stable")
    counts = np.bincount(key, minlength=N_CORES * NT * NG)
    quota = counts.reshape(N_CORES, NT, NG).max(axis=0)
    quota = ((quota + 127) // 128) * 128  # round up to whole 128-slot blocks

    calls, blk_tile, call_off_flat, slot_total = _build_schedule(
        quota, NT, NG)

    # slot position of each edge inside its core's stream
    csum = np.zeros(N_CORES * NT * NG + 1, np.int64)
    np.cumsum(counts, out=csum[1:])
    sorted_key = key[order]
    rank = np.arange(E, dtype=np.int64) - csum[sorted_key]
    tg = t_of[order] * NG + g_of[order]
    slot = call_off_flat[tg] + rank
    core = c_of[order]

    idx_arr = np.zeros((N_CORES, slot_total), np.int16)  # pad -> row 0
    dl_arr = np.full((N_CORES, slot_total), PAD_DSTLOC, np.float32)
    idx_arr[core, slot] = srcg[order]
    dl_arr[core, slot] = d_of[order]

    # per-section wrapping: idx wrapped [16, Q/16] (replicated to 128
    # parts), dstloc wrapped [128, Q/128]
    idxcols = slot_total // 16
    nb = slot_total // 128
    idx_sb = np.zeros((N_CORES, 16, idxcols), np.int16)
    dl_sb = np.full((N_CORES, 128, nb), PAD_DSTLOC, np.float32)
    for tgi in range(NT * NG):
        q = int(quota.reshape(-1)[tgi])
        if q == 0:
            continue
        o = int(call_off_flat[tgi])
        seg = idx_arr[:, o:o + q].reshape(N_CORES, q // 16, 16)
        idx_sb[:, :, o // 16:(o + q) // 16] = seg.transpose(0, 2, 1)
        dseg = dl_arr[:, o:o + q].reshape(N_CORES, q // 128, 128)
        dl_sb[:, :, o // 128:(o + q) // 128] = dseg.transpose(0, 2, 1)
    idx_sb = np.tile(idx_sb, (1, 8, 1))  # replicate to 128 partitions

    # dense inputs
    xp = np.zeros((NPAD, D_IN), np.float32)
    xp[:N] = x
    xT = np.ascontiguousarray(xp.T)  # [D_IN, NPAD]

    ntile = NPAD // 128
    dinv_nodes = np.ascontiguousarray(
        dinv_pad.reshape(ntile, 128).T)  # [128, ntile]
    dinv_dst = np.stack([dinv_nodes[:, c * NT:(c + 1) * NT]
                         for c in range(N_CORES)])  # [8, 128, NT]

    iota = np.tile(np.arange(128), (128, 1)).astype(NP_BF16)
    ident = np.eye(128, dtype=np.float32)
    b1bc = np.ascontiguousarray(
        np.broadcast_to(np.asarray(b1, np.float32), (128, D_HID)))
    b2bc = np.ascontiguousarray(
        np.broadcast_to(np.asarray(b2, np.float32), (128, D_OUT)))

    common = dict(xT=xT.astype(NP_BF16), W1=np.asarray(W1, NP_BF16),
                  W2=np.asarray(W2, np.float32), b1bc=b1bc, b2bc=b2bc,
                  iota=iota, ident=ident, dinv_nodes=dinv_nodes)
    in_maps = []
    for c in range(N_CORES):
        m = dict(common)
        m["dinv_dst"] = np.ascontiguousarray(dinv_dst[c])
        m["idx_sb"] = np.ascontiguousarray(idx_sb[c])
        m["dstloc"] = np.ascontiguousarray(dl_sb[c])
        in_maps.append(m)

    meta = dict(quota=quota, idxcols=idxcols, nb=nb, calls=calls,
                blk_tile=blk_tile)
    return in_maps, meta


# --------------------------------------------------------------------------
# Device program
# --------------------------------------------------------------------------

def build_program(cfg, meta, with_collective=True, phases=(1, 2, 3)):
    NPAD, NT, NG, GR, SUP = (cfg["NPAD"], cfg["NT"], cfg["NG"], cfg["GR"],
                             cfg["SUP"])
    D_IN, D_HID, D_OUT = cfg["D_IN"], cfg["D_HID"], cfg["D_OUT"]
    SHARD = cfg["SHARD"]
    quota = meta["quota"]
    idxcols, nb = meta["idxcols"], meta["nb"]
    calls, blk_tile = meta["calls"], meta["blk_tile"]
    ntile = NPAD // 128
    KI = D_IN // 128   # k-chunks for layer-1 matmul
    KH = D_HID // 128  # k-chunks for W2 matmul
    CMAX = QMAX // 128
    D_L2 = 2 * D_OUT  # layer-2 table row: 64 bf16 data + 64 bf16 zeros

    # first/last block of each tile (accumulation start/stop flags)
    first_blk = {}
    last_blk = {}
    for i, t in enumerate(blk_tile):
        first_blk.setdefault(t, i)
        last_blk[t] = i

    nc = bacc.Bacc("TRN2", target_bir_lowering=False, debug=False,
                   num_devices=N_CORES)

    xT_d = nc.dram_tensor("xT", [D_IN, NPAD], BF16, kind="ExternalInput")
    W1_d = nc.dram_tensor("W1", [D_IN, D_HID], BF16, kind="ExternalInput")
    W2_d = nc.dram_tensor("W2", [D_HID, D_OUT], F32, kind="ExternalInput")
    b1_d = nc.dram_tensor("b1bc", [128, D_HID], F32, kind="ExternalInput")
    b2_d = nc.dram_tensor("b2bc", [128, D_OUT], F32, kind="ExternalInput")
    iota_d = nc.dram_tensor("iota", [128, 128], BF16, kind="ExternalInput")
    ident_d = nc.dram_tensor("ident", [128, 128], F32, kind="ExternalInput")
    dinvn_d = nc.dram_tensor("dinv_nodes", [128, ntile], F32,
                             kind="ExternalInput")
    dinvd_d = nc.dram_tensor("dinv_dst", [128, NT], F32, kind="ExternalInput")
    idx_d = nc.dram_tensor("idx_sb", [128, idxcols], I16, kind="ExternalInput")
    dl_d = nc.dram_tensor("dstloc", [128, nb], F32, kind="ExternalInput")
    out_d = nc.dram_tensor("out", [SHARD, D_OUT], F32, kind="ExternalOutput")

    with tile.TileContext(nc) as tc:
        with (
            tc.tile_pool(name="const", bufs=1) as const,
            tc.tile_pool(name="dram", bufs=1, space="DRAM") as dram,
        ):
            h1p_g = [dram.tile([GR, D_HID], BF16, name=f"h1p{g}")
                     for g in range(NG)]
            h2own = dram.tile([SHARD, D_L2], BF16)
            h2full = dram.tile([NPAD, D_L2], BF16, addr_space="Shared")

            w1_sb = const.tile([128, KI, D_HID], BF16)
            for k in range(KI):
                nc.sync.dma_start(out=w1_sb[:, k, :],
                                  in_=W1_d.ap()[k * 128:(k + 1) * 128, :])
            w2_sb = const.tile([128, KH, D_OUT], F32)
            for k in range(KH):
                nc.sync.dma_start(out=w2_sb[:, k, :],
                                  in_=W2_d.ap()[k * 128:(k + 1) * 128, :])
            iota_sb = const.tile([128, 128], BF16)
            nc.sync.dma_start(out=iota_sb[:], in_=iota_d.ap())
            ident_sb = const.tile([128, 128], F32)
            nc.sync.dma_start(out=ident_sb[:], in_=ident_d.ap())
            b1_sb = const.tile([128, D_HID], F32)
            nc.sync.dma_start(out=b1_sb[:], in_=b1_d.ap())
            b2_sb = const.tile([128, D_OUT], F32)
            nc.sync.dma_start(out=b2_sb[:], in_=b2_d.ap())
            dinvn_sb = const.tile([128, ntile], F32)
            nc.sync.dma_start(out=dinvn_sb[:], in_=dinvn_d.ap())
            dinvd_sb = const.tile([128, NT], F32)
            nc.sync.dma_start(out=dinvd_sb[:], in_=dinvd_d.ap())
            idx_sb = const.tile([128, idxcols], I16)
            nc.sync.dma_start(out=idx_sb[:], in_=idx_d.ap())
            dl_sb = const.tile([128, nb], F32)
            nc.sync.dma_start(out=dl_sb[:], in_=dl_d.ap())

            # ---------------- phase 1: h1p = dinv * (x @ W1) --------------
            nsup = NPAD // SUP if 1 in phases else 0
            nsub = SUP // 128
            assert GR % SUP == 0
            sup_per_g = GR // SUP
            xT_r = xT_d.ap().rearrange("(k p) n -> p k n", p=128)
            h1p_g_r = [t.rearrange("(s u p) f -> s p u f", p=128, u=nsub)
                       for t in h1p_g]
            with (
                tc.tile_pool(name="p1x", bufs=4) as p1x,
                tc.tile_pool(name="p1o", bufs=4) as p1o,
                tc.tile_pool(name="p1ps", bufs=4, space="PSUM") as p1ps,
            ):
                for s in range(nsup):
                    xt = p1x.tile([128, KI, SUP], BF16, tag="xt")
                    nc.sync.dma_start(
                        out=xt[:], in_=xT_r[:, :, s * SUP:(s + 1) * SUP])
                    ot = p1o.tile([128, nsub, D_HID], BF16, tag="h1o")
                    for u in range(nsub):
                        ps = p1ps.tile([128, D_HID], F32, tag="p1")
                        for k in range(KI):
                            nc.tensor.matmul(
                                ps[:],
                                xt[:, k, u * 128:(u + 1) * 128],
                                w1_sb[:, k, :],
                                start=(k == 0), stop=(k == KI - 1))
                        gt = s * nsub + u
                        nc.scalar.activation(
                            ot[:, u, :], ps[:], AF.Copy,
                            scale=dinvn_sb[:, gt:gt + 1])
                    nc.sync.dma_start(
                        out=h1p_g_r[s // sup_per_g][s % sup_per_g],
                        in_=ot[:])

            # ---------------- phase 2: layer-1 agg + h2p shard ------------
            h2own_r = h2own.rearrange("(t p) f -> t p f", p=128)

            def agg_phase(table, elem, rhs_w, epilogue, mtag, stag, ptag):
                """Gather + one-hot-S + matmul accumulation over the
                precomputed quad-spanning call schedule."""
                blk = 0
                psums = {}
                with (
                    tc.tile_pool(name=mtag, bufs=10) as mpool,
                    tc.tile_pool(name=stag, bufs=8) as spool,
                    tc.tile_pool(name=ptag, bufs=6, space="PSUM") as apsum,
                    tc.tile_pool(name=ptag + "ep", bufs=3) as eppool,
                    tc.tile_pool(name=ptag + "ep2", bufs=1,
                                 space="PSUM") as eppsum,
                ):
                    for g, o, q in calls:
                        ncols = q // 128
                        mt = mpool.tile([128, CMAX, elem], BF16, tag="m")
                        nc.gpsimd.dma_gather(
                            mt[:, :ncols, :],
                            table(g),
                            idx_sb[:, o // 16:(o + q) // 16],
                            q, q, elem)
                        for j in range(ncols):
                            t = blk_tile[blk]
                            if blk == first_blk[t]:
                                psums[t] = apsum.tile(
                                    [128, rhs_w], F32, tag="agg",
                                    name="aggps")
                            st = spool.tile([128, 128], BF16, tag="s",
                                            name="stile")
                            nc.vector.tensor_scalar(
                                st[:], iota_sb[:], dl_sb[:, blk:blk + 1],
                                None, ALU.is_equal)
                            nc.tensor.matmul(
                                psums[t][:], st[:], mt[:, j, :rhs_w],
                                start=(blk == first_blk[t]),
                                stop=(blk == last_blk[t]))
                            if blk == last_blk[t]:
                                epilogue(t, psums.pop(t), eppool, eppsum)
                            blk += 1

            def epi1(t, ps, eppool, eppsum):
                o1 = eppool.tile([128, D_HID], F32, tag="o1")
                nc.vector.tensor_scalar(o1[:], ps[:], dinvd_sb[:, t:t + 1],
                                        None, ALU.mult)
                nc.vector.tensor_tensor(o1[:], o1[:], b1_sb[:], ALU.add)
                nc.scalar.activation(o1[:], o1[:], AF.Relu)
                tsb = eppool.tile([128, KH, 128], F32, tag="tsb")
                for k in range(KH):
                    tp = eppsum.tile([128, 128], F32, tag="tr")
                    nc.tensor.transpose(tp[:], o1[:, k * 128:(k + 1) * 128],
                                        ident_sb[:])
                    nc.vector.tensor_copy(tsb[:, k, :], tp[:])
                h2ps = eppsum.tile([128, D_OUT], F32, tag="h2")
                for k in range(KH):
                    nc.tensor.matmul(h2ps[:], tsb[:, k, :], w2_sb[:, k, :],
                                     start=(k == 0), stop=(k == KH - 1))
                h2sb = eppool.tile([128, D_L2], BF16, tag="h2sb")
                nc.vector.tensor_scalar(h2sb[:, :D_OUT], h2ps[:],
                                        dinvd_sb[:, t:t + 1], None, ALU.mult)
                nc.vector.memset(h2sb[:, D_OUT:], 0.0)
                nc.sync.dma_start(out=h2own_r[t], in_=h2sb[:])

            if 2 in phases:
                agg_phase(lambda g: h1p_g[g][:, :], D_HID, D_HID, epi1,
                          "m1", "s1", "ag1")

            # ---------------- AllGather h2 shards -------------------------
            if with_collective and 2 in phases:
                nc.gpsimd.collective_compute(
                    "AllGather", ALU.bypass,
                    replica_groups=[list(range(N_CORES))],
                    ins=[h2own.opt()], outs=[h2full.opt()])

            # ---------------- phase 3: layer-2 agg + log_softmax ----------
            out_r = out_d.ap().rearrange("(t p) f -> t p f", p=128)

            def epi2(t, ps, eppool, eppsum):
                t0 = eppool.tile([128, D_OUT], F32, tag="t0")
                nc.vector.tensor_scalar(t0[:], ps[:], dinvd_sb[:, t:t + 1],
                                        None, ALU.mult)
                nc.vector.tensor_tensor(t0[:], t0[:], b2_sb[:], ALU.add)
                nm = eppool.tile([128, 1], F32, tag="nm")
                nc.vector.tensor_reduce(nm[:], t0[:], mybir.AxisListType.X,
                                        ALU.max, negate=True)
                et = eppool.tile([128, D_OUT], F32, tag="et")
                se = eppool.tile([128, 1], F32, tag="se")
                nc.scalar.activation(et[:], t0[:], AF.Exp, bias=nm[:],
                                     accum_out=se[:])
                ls = eppool.tile([128, 1], F32, tag="ls")
                nc.scalar.activation(ls[:], se[:], AF.Ln)
                ot = eppool.tile([128, D_OUT], F32, tag="ot")
                nc.vector.tensor_scalar(ot[:], t0[:], nm[:], ls[:],
                                        ALU.add, ALU.subtract)
                nc.sync.dma_start(out=out_r[t], in_=ot[:])

            if 3 in phases:
                agg_phase(lambda g: h2full[g * GR:(g + 1) * GR, :], D_L2,
                          D_OUT, epi2, "m2", "s2", "ag2")

    nc.compile()
    return nc


# --------------------------------------------------------------------------
# Entry point
# --------------------------------------------------------------------------

def kernel(x, edge_index, W1, b1, W2, b2):
    cfg = FULL_CFG
    in_maps, meta = preprocess(x, edge_index, W1, b1, W2, b2, cfg)
    nc = build_program(cfg, meta)
    res = run_bass_kernel_spmd(nc, in_maps, core_ids=list(range(N_CORES)))
    shards = [res.results[c]["out"] for c in range(N_CORES)]
    full = np.concatenate(shards, axis=0)
    return full[:cfg["N"]].astype(np.float32)
